# revision 18
# baseline (speedup 1.0000x reference)
"""Multi-head attention (B=2, T=2048, H=8, K=128) on 8 TRN2 NeuronCores.

Sharding: tensor-parallel over heads - core c owns head c for both batches.
Each core computes its head's (unnormalized) attention output projected
through its slice of Wu; the host divides by the shipped per-token softmax
denominators, sums the 8 partials and adds the bias.

Per-core dataflow (features on partitions, tokens on the free axis):

  host:  xt = bf16(x^T) [128 i, 4096 t]; W slices pre-cast to bf16.
  qt/kt = Wq/Wk^T  xt     [128 d, 4096]  bf16  (W stationary, xt moving)
  V     = (xt_c)^T Wv     [128 keys, 128 d] per 128-token chunk - produced
          directly in [keys, dims] layout by making the xt chunk stationary;
          cast to fp8e4 (DVE) into DoubleRow slabs.
  per 1024-token block, per 128-key chunk c (3-deep software pipeline):
      S^T_c = kt_c qt       [128, 1024] PSUM fp32        (bf16 matmul)
      E_c   = exp(S^T_c/sqrt(128)) -> fp8e4 SBUF
              even chunks + every 8th odd: ACT Exp (direct fp8 output)
              other odd chunks: DVE Schraudolph - int8(S*a+b) IS the fp8
              bit pattern of exp (a=8 log2(e)/sqrt(128), b=56-0.3)
    per pair p (chunks 2p, 2p+1), fp8 DoubleRow matmul (0.5 cyc/row):
      Y^T += V_pair^T E_pair     [128, 1024] PSUM
    block tail (deferred into the next block's first chunks so no engine
    queue stalls at the boundary):
      sumexp = ones8^T E_pair, burst over all 8 pairs   (fp8 DR)
      y_bf = bf16(Y^T) (DVE); out^T_blk = Wu^T y_bf (bf16);
      out^T -> SBUF (DVE) -> DRAM; sumexp row 0 -> SBUF (ACT) -> DRAM.

Host: out = sum_c (out_c / sumexp_c)^T + bu, reshaped to (2, 2048, 128).

fp8 error budget (simulated offline vs fp64 truth): E+V fp8 with fp8-domain
Schraudolph on 7/16 of chunks -> rel err ~1.4e-2 (gate: 2e-2); softmax
normalization absorbs most of the correlated low-precision-E error.

PSUM (8 banks): ps pool 3x[128,1024]f32 (S tiles / sum burst / out proj,
rotating) = 6 banks, ps_y 1x[128,1024] (Y accumulator) = 2 banks.
"""

import sys

import numpy as np

if "/opt/trn_rl_repo" not in sys.path:
    sys.path.insert(0, "/opt/trn_rl_repo")

import ml_dtypes

B, T, K, H = 2, 2048, 128, 8
BT = B * T              # 4096 tokens over both batches
TB = 1024               # token block
NBLK = BT // TB         # 4
NCH = T // 128          # 16 key chunks per batch
NCORES = 8
SCALE = 1.0 / np.sqrt(np.float32(K))
SCHR_A = float(SCALE * np.log2(np.e) * 8.0)
SCHR_B = 56.0 - 0.3

_compiled = None


def _is_dve_chunk(c):
    # DVE (Schraudolph) takes odd chunks, ACT takes even -> no double-ACT
    # pairs to stall the pipeline
    return c % 2 == 1


def _build():
    import concourse.mybir as mybir
    import concourse.tile as tile
    from concourse import bacc

    f32 = mybir.dt.float32
    bf16 = mybir.dt.bfloat16
    fp8 = mybir.dt.float8e4
    i8 = mybir.dt.int8
    Exp = mybir.ActivationFunctionType.Exp
    Copy = mybir.ActivationFunctionType.Copy
    DR = mybir.MatmulPerfMode.DoubleRow

    nc = bacc.Bacc(
        "TRN2",
        target_bir_lowering=False,
        debug=False,
        enable_asserts=False,
        num_devices=NCORES,
    )

    xt_d = nc.dram_tensor("xt", [K, BT], bf16, kind="ExternalInput").ap()
    wq_d = nc.dram_tensor("wq", [K, K], bf16, kind="ExternalInput").ap()
    wk_d = nc.dram_tensor("wk", [K, K], bf16, kind="ExternalInput").ap()
    wv_d = nc.dram_tensor("wv", [K, K], bf16, kind="ExternalInput").ap()
    wu_d = nc.dram_tensor("wu", [K, K], bf16, kind="ExternalInput").ap()
    out_d = nc.dram_tensor("out", [K, BT], f32, kind="ExternalOutput").ap()
    sums_d = nc.dram_tensor("sums", [1, BT], f32, kind="ExternalOutput").ap()

    with tile.TileContext(nc) as tc:
        from contextlib import ExitStack

        with ExitStack() as ctx:
            const = ctx.enter_context(tc.tile_pool(name="const", bufs=1))
            big = ctx.enter_context(tc.tile_pool(name="big", bufs=1))
            e8p = ctx.enter_context(tc.tile_pool(name="e8p", bufs=13))
            work = ctx.enter_context(tc.tile_pool(name="work", bufs=2))
            ps = ctx.enter_context(tc.tile_pool(name="ps", bufs=3, space="PSUM"))
            ps_y = ctx.enter_context(tc.tile_pool(name="ps_y", bufs=1, space="PSUM"))

            # inputs: xt chunk 0 + weights first (gate the projections), xt
            # rest on the idle gpsimd queue so the DMAs issue in parallel
            xt_c = [big.tile([128, 1024], bf16, tag=f"xt{c}", name=f"xt{c}")
                    for c in range(4)]
            wq_sb = const.tile([128, 128], bf16, tag="wq")
            wk_sb = const.tile([128, 128], bf16, tag="wk")
            wv_sb = const.tile([128, 128], bf16, tag="wv")
            wu_sb = const.tile([128, 128], bf16, tag="wu")
            nc.sync.dma_start(xt_c[0][:], xt_d[:, 0:1024])
            nc.sync.dma_start(wq_sb[:], wq_d[:])
            nc.scalar.dma_start(wk_sb[:], wk_d[:])
            nc.scalar.dma_start(wv_sb[:], wv_d[:])
            nc.scalar.dma_start(wu_sb[:], wu_d[:])
            for c in range(1, 4):
                nc.gpsimd.dma_start(xt_c[c][:], xt_d[:, 1024 * c : 1024 * (c + 1)])

            ones8 = const.tile([128, 2, 128], fp8, tag="ones8")
            nc.gpsimd.memset(ones8[:], 1.0)
            sums_sb = const.tile([1, BT], f32, tag="sums_sb")

            def col(chunks, c, w):
                i, off = divmod(c, 1024)
                return chunks[i][:, off : off + w]

            # Q^T / K^T projections: W stationary, xt moving -> psum -> bf16
            # (cast on ACT: DVE is loaded with Schraudolph exp later).
            # Only the pieces needed by attention block 0's first pairs are
            # emitted up-front; the rest interleave into the attention loop.
            qt_c = [big.tile([128, 1024], bf16, tag=f"qt{c}", name=f"qt{c}")
                    for c in range(4)]
            kt_c = [big.tile([128, 1024], bf16, tag=f"kt{c}", name=f"kt{c}")
                    for c in range(4)]
            vt_c = [big.tile([128, 1024], bf16, tag=f"vt{c}", name=f"vt{c}")
                    for c in range(4)]
            # w8_c[i] = fp8(V_chunk @ Wu) for key chunks 4i..4i+3: folding Wu
            # into V lets the attention matmul produce out^T directly
            w8_c = [big.tile([128, 4, 128], fp8, tag=f"w8_{i}", name=f"w8_{i}")
                    for i in range(8)]

            def pcast(eng, dst, src):
                if eng == "act":
                    nc.scalar.activation(dst, src, Copy)
                else:
                    nc.vector.tensor_copy(dst, src)

            def emit_qk(w_sb, dst, c, eng, split=False):
                pp = ps.tile([128, 1024], f32, tag="s", name="pp")
                for g in range(2):
                    nc.tensor.matmul(
                        pp[:, 512 * g : 512 * (g + 1)],
                        w_sb[:],
                        xt_c[c][:, 512 * g : 512 * (g + 1)],
                        start=True,
                        stop=True,
                    )
                    if split:
                        pcast(eng, dst[c][:, 512 * g : 512 * (g + 1)],
                              pp[:, 512 * g : 512 * (g + 1)])
                if not split:
                    pcast(eng, dst[c][:], pp[:])

            def emit_w8(i, eng):
                pw = ps.tile([128, 512], f32, tag="s", name="pw")
                for j in range(4):
                    ch = 4 * i + j
                    nc.tensor.matmul(
                        pw[:, 128 * j : 128 * (j + 1)],
                        col(vt_c, 128 * ch, 128),
                        wu_sb[:],
                        start=True,
                        stop=True,
                    )
                pcast(eng, w8_c[i][:],
                      pw[:].rearrange("p (a b) -> p a b", a=4))

            emit_qk(wq_sb, qt_c, 0, "act", split=True)
            emit_qk(wk_sb, kt_c, 0, "act", split=True)
            emit_qk(wv_sb, vt_c, 0, "vec")
            emit_w8(0, "act")
            emit_w8(1, "vec")
            emit_qk(wq_sb, qt_c, 1, "vec")
            emit_qk(wk_sb, kt_c, 1, "act")
            emit_qk(wv_sb, vt_c, 1, "vec")
            emit_w8(2, "act")
            emit_w8(3, "vec")
            emit_qk(wq_sb, qt_c, 2, "act")
            emit_qk(wk_sb, kt_c, 2, "vec")
            emit_qk(wv_sb, vt_c, 2, "act")
            emit_w8(4, "vec")
            emit_w8(5, "act")
            emit_qk(wq_sb, qt_c, 3, "vec")
            emit_qk(wk_sb, kt_c, 3, "act")
            emit_qk(wv_sb, vt_c, 3, "vec")
            emit_w8(6, "act")
            emit_w8(7, "vec")
            proj_rest = []

            # attention: 4 blocks x 16 chunks, 3-deep software pipeline with
            # block tails deferred into the next block's first chunks
            chunks = [(blk, c) for blk in range(NBLK) for c in range(NCH)]
            DEPTH = 3

            def s_matmul(blk, c):
                b = blk // 2
                scol = b * T + c * 128
                tcol = blk * TB
                sp = ps.tile([128, TB], f32, tag="s", name=f"ps_{blk}_{c}")
                for g in range(2):
                    nc.tensor.matmul(
                        sp[:, 512 * g : 512 * (g + 1)],
                        col(kt_c, scol, 128),
                        col(qt_c, tcol + 512 * g, 512),
                        start=True,
                        stop=True,
                    )
                return sp

            def tail(blk, step, st):
                # deferred tail of block `blk`; step advances with the next
                # block's chunks so no engine queue blocks at the boundary
                if step == 0:
                    out_sb = work.tile([128, TB], f32, tag="outsb",
                                       name=f"osb{blk}")
                    nc.vector.tensor_copy(out_sb[:], st["py"][:])
                    nc.sync.dma_start(
                        out_d[:, blk * TB : (blk + 1) * TB], out_sb[:])
                elif step == 1:
                    psumt = ps.tile([128, TB], f32, tag="s", name=f"sum{blk}")
                    for pair in range(8):
                        for g in range(2):
                            sl = slice(512 * g, 512 * (g + 1))
                            nc.tensor.matmul(
                                psumt[:, sl],
                                ones8[:],
                                st["e8"][pair][:, :, sl],
                                start=(pair == 0),
                                stop=(pair == 7),
                                perf_mode=DR,
                            )
                    st["psumt"] = psumt
                elif step == 2:
                    nc.scalar.activation(
                        sums_sb[0:1, blk * TB : (blk + 1) * TB],
                        st["psumt"][0:1, :], Copy)

            pend = [s_matmul(*chunks[i]) for i in range(DEPTH)]
            prev_tail = None
            st = None
            pend_ydr = []

            def flush_one_ydr():
                yst, pair, vp, voff, e8t = pend_ydr.pop(0)
                for g in range(2):
                    sl = slice(512 * g, 512 * (g + 1))
                    nc.tensor.matmul(
                        yst["py"][:, sl],
                        vp[:, voff : voff + 2, :],
                        e8t[:, :, sl],
                        start=(pair == 0),
                        stop=(pair == 7),
                        perf_mode=DR,
                        skip_group_check=True,
                    )

            def flush_ydr():
                while pend_ydr:
                    flush_one_ydr()
            for ci, (blk, c) in enumerate(chunks):
                if c == 0:
                    st = {"py": ps_y.tile([128, TB], f32, tag="y",
                                          name=f"py{blk}"),
                          "e8": []}
                sp = pend.pop(0)
                if ci + DEPTH < len(chunks):
                    pend.append(s_matmul(*chunks[ci + DEPTH]))
                if c % 2 == 0:
                    st["e8"].append(e8p.tile([128, 2, TB], fp8, tag="e8",
                                             name=f"e8_{blk}_{c // 2}"))
                e8_pair = st["e8"][c // 2]
                if _is_dve_chunk(c):
                    nc.vector.tensor_scalar(
                        e8_pair[:, c % 2, :].bitcast(i8),
                        sp[:],
                        SCHR_A,
                        SCHR_B,
                        mybir.AluOpType.mult,
                        mybir.AluOpType.add,
                    )
                else:
                    nc.scalar.activation(
                        e8_pair[:, c % 2, :], sp[:], Exp, scale=float(SCALE)
                    )
                if c % 2 == 1:
                    pair = c // 2
                    b = blk // 2
                    vp = w8_c[(b * NCH + c - 1) // 4]
                    voff = ((c - 1) % 4) // 2 * 2
                    pend_ydr.append((st, pair, vp, voff, e8_pair))
                else:
                    while len(pend_ydr) > 2:
                        flush_one_ydr()
                if c % 2 == 1 and proj_rest:
                    proj_rest.pop(0)()
                if prev_tail is not None and c in (0, 1, 2):
                    if c == 0:
                        flush_ydr()
                    tail(prev_tail[0], c, prev_tail[1])
                    if c == 2:
                        prev_tail = None
                if c == NCH - 1:
                    prev_tail = (blk, st)
            # last block's tail: out copy + DMA first, sum burst last
            flush_ydr()
            lblk, lst = prev_tail
            out_sb = work.tile([128, TB], f32, tag="outsb", name="osbL")
            for g in range(2):
                sl = slice(512 * g, 512 * (g + 1))
                nc.vector.tensor_copy(out_sb[:, sl], lst["py"][:, sl])
                nc.sync.dma_start(out_d[:, lblk * TB + 512 * g :
                                        lblk * TB + 512 * (g + 1)],
                                  out_sb[:, sl])
            psumt = ps.tile([128, TB], f32, tag="s", name="sumL")
            for g in range(2):
                sl = slice(512 * g, 512 * (g + 1))
                for pair in range(8):
                    nc.tensor.matmul(
                        psumt[:, sl],
                        ones8[:],
                        lst["e8"][pair][:, :, sl],
                        start=(pair == 0),
                        stop=(pair == 7),
                        perf_mode=DR,
                    )
                nc.scalar.activation(
                    sums_sb[0:1, lblk * TB + 512 * g : lblk * TB + 512 * (g + 1)],
                    psumt[0:1, sl], Copy)
                nc.sync.dma_start(
                    sums_d[0:1, lblk * TB + 512 * g : lblk * TB + 512 * (g + 1)],
                    sums_sb[0:1, lblk * TB + 512 * g : lblk * TB + 512 * (g + 1)])
            nc.sync.dma_start(sums_d[0:1, 0 : lblk * TB],
                              sums_sb[0:1, 0 : lblk * TB])

    nc.compile()
    return nc


def _get_nc():
    global _compiled
    if _compiled is None:
        _compiled = _build()
    return _compiled


def kernel(x, Wq, Wk, Wv, Wu, bu, **_run_kwargs):
    from concourse.bass_utils import run_bass_kernel_spmd

    nc = _get_nc()

    bf = ml_dtypes.bfloat16
    x = np.asarray(x, dtype=np.float32).reshape(BT, K)
    xt = np.ascontiguousarray(x.T.astype(bf))
    Wq = np.asarray(Wq, dtype=np.float32)
    Wk = np.asarray(Wk, dtype=np.float32)
    Wv = np.asarray(Wv, dtype=np.float32)
    Wu = np.asarray(Wu, dtype=np.float32)
    bu = np.asarray(bu, dtype=np.float32)

    in_maps = []
    for c in range(NCORES):
        sl = slice(c * K, (c + 1) * K)
        in_maps.append(
            {
                "xt": xt,
                "wq": np.ascontiguousarray(Wq[:, sl].astype(bf)),
                "wk": np.ascontiguousarray(Wk[:, sl].astype(bf)),
                "wv": np.ascontiguousarray(Wv[:, sl].astype(bf)),
                "wu": np.ascontiguousarray(Wu[sl, :].astype(bf)),
            }
        )

    res = run_bass_kernel_spmd(nc, in_maps, list(range(NCORES)), **_run_kwargs)

    out = np.zeros((BT, K), dtype=np.float64)
    for c in range(NCORES):
        o = np.asarray(res.results[c]["out"], dtype=np.float64)   # [128, 4096]
        s = np.asarray(res.results[c]["sums"], dtype=np.float64).reshape(BT)
        out += (o / s[None, :]).T
    out += bu[None, :].astype(np.float64)
    result = out.astype(np.float32).reshape(B, T, K)
    if _run_kwargs:
        return result, res
    return result


# revision 20
# speedup vs baseline: 1.0087x; 1.0087x over previous
"""Multi-head attention (B=2, T=2048, H=8, K=128) on 8 TRN2 NeuronCores.

Sharding: tensor-parallel over heads - core c owns head c for both batches.
Each core computes its head's (unnormalized) attention output projected
through its slice of Wu; the host divides by the shipped per-token softmax
denominators, sums the 8 partials and adds the bias.

Per-core dataflow (features on partitions, tokens on the free axis):

  host:  xt = bf16(x^T) [128 i, 4096 t]; W slices pre-cast to bf16.
  qt/kt = Wq/Wk^T  xt     [128 d, 4096]  bf16  (W stationary, xt moving)
  V     = (xt_c)^T Wv     [128 keys, 128 d] per 128-token chunk - produced
          directly in [keys, dims] layout by making the xt chunk stationary;
          cast to fp8e4 (DVE) into DoubleRow slabs.
  per 1024-token block, per 128-key chunk c (3-deep software pipeline):
      S^T_c = kt_c qt       [128, 1024] PSUM fp32        (bf16 matmul)
      E_c   = exp(S^T_c/sqrt(128)) -> fp8e4 SBUF
              even chunks + every 8th odd: ACT Exp (direct fp8 output)
              other odd chunks: DVE Schraudolph - int8(S*a+b) IS the fp8
              bit pattern of exp (a=8 log2(e)/sqrt(128), b=56-0.3)
    per pair p (chunks 2p, 2p+1), fp8 DoubleRow matmul (0.5 cyc/row):
      Y^T += V_pair^T E_pair     [128, 1024] PSUM
    block tail (deferred into the next block's first chunks so no engine
    queue stalls at the boundary):
      sumexp = ones8^T E_pair, burst over all 8 pairs   (fp8 DR)
      y_bf = bf16(Y^T) (DVE); out^T_blk = Wu^T y_bf (bf16);
      out^T -> SBUF (DVE) -> DRAM; sumexp row 0 -> SBUF (ACT) -> DRAM.

Host: out = sum_c (out_c / sumexp_c)^T + bu, reshaped to (2, 2048, 128).

fp8 error budget (simulated offline vs fp64 truth): E+V fp8 with fp8-domain
Schraudolph on 7/16 of chunks -> rel err ~1.4e-2 (gate: 2e-2); softmax
normalization absorbs most of the correlated low-precision-E error.

PSUM (8 banks): ps pool 3x[128,1024]f32 (S tiles / sum burst / out proj,
rotating) = 6 banks, ps_y 1x[128,1024] (Y accumulator) = 2 banks.
"""

import sys

import numpy as np

if "/opt/trn_rl_repo" not in sys.path:
    sys.path.insert(0, "/opt/trn_rl_repo")

import ml_dtypes

B, T, K, H = 2, 2048, 128, 8
BT = B * T              # 4096 tokens over both batches
TB = 1024               # token block
NBLK = BT // TB         # 4
NCH = T // 128          # 16 key chunks per batch
NCORES = 8
SCALE = 1.0 / np.sqrt(np.float32(K))
SCHR_A = float(SCALE * np.log2(np.e) * 8.0)
SCHR_B = 56.0 - 0.3

_compiled = None


def _is_dve_chunk(c):
    # DVE (Schraudolph) takes odd chunks, ACT takes even -> no double-ACT
    # pairs to stall the pipeline
    return c % 2 == 1


def _build():
    import concourse.mybir as mybir
    import concourse.tile as tile
    from concourse import bacc

    f32 = mybir.dt.float32
    bf16 = mybir.dt.bfloat16
    fp8 = mybir.dt.float8e4
    i8 = mybir.dt.int8
    Exp = mybir.ActivationFunctionType.Exp
    Copy = mybir.ActivationFunctionType.Copy
    DR = mybir.MatmulPerfMode.DoubleRow

    nc = bacc.Bacc(
        "TRN2",
        target_bir_lowering=False,
        debug=False,
        enable_asserts=False,
        num_devices=NCORES,
    )

    xt_d = nc.dram_tensor("xt", [K, BT], bf16, kind="ExternalInput").ap()
    wq_d = nc.dram_tensor("wq", [K, K], bf16, kind="ExternalInput").ap()
    wk_d = nc.dram_tensor("wk", [K, K], bf16, kind="ExternalInput").ap()
    wv_d = nc.dram_tensor("wv", [K, K], bf16, kind="ExternalInput").ap()
    wu_d = nc.dram_tensor("wu", [K, K], bf16, kind="ExternalInput").ap()
    out_d = nc.dram_tensor("out", [K, BT], f32, kind="ExternalOutput").ap()
    sums_d = nc.dram_tensor("sums", [1, BT], f32, kind="ExternalOutput").ap()

    with tile.TileContext(nc) as tc:
        from contextlib import ExitStack

        with ExitStack() as ctx:
            const = ctx.enter_context(tc.tile_pool(name="const", bufs=1))
            big = ctx.enter_context(tc.tile_pool(name="big", bufs=1))
            e8p = ctx.enter_context(tc.tile_pool(name="e8p", bufs=11))
            work = ctx.enter_context(tc.tile_pool(name="work", bufs=2))
            ps = ctx.enter_context(tc.tile_pool(name="ps", bufs=3, space="PSUM"))
            ps_y = ctx.enter_context(tc.tile_pool(name="ps_y", bufs=1, space="PSUM"))

            # inputs: xt chunk 0 + weights first (gate the projections), xt
            # rest on the idle gpsimd queue so the DMAs issue in parallel
            xt_c = [big.tile([128, 1024], bf16, tag=f"xt{c}", name=f"xt{c}")
                    for c in range(4)]
            wq_sb = const.tile([128, 128], bf16, tag="wq")
            wk_sb = const.tile([128, 128], bf16, tag="wk")
            wv_sb = const.tile([128, 128], bf16, tag="wv")
            wu_sb = const.tile([128, 128], bf16, tag="wu")
            nc.sync.dma_start(xt_c[0][:], xt_d[:, 0:1024])
            nc.sync.dma_start(wq_sb[:], wq_d[:])
            nc.scalar.dma_start(wk_sb[:], wk_d[:])
            nc.scalar.dma_start(wv_sb[:], wv_d[:])
            nc.scalar.dma_start(wu_sb[:], wu_d[:])
            for c in range(1, 4):
                nc.gpsimd.dma_start(xt_c[c][:], xt_d[:, 1024 * c : 1024 * (c + 1)])

            ones8 = const.tile([128, 2, 128], fp8, tag="ones8")
            nc.gpsimd.memset(ones8[:], 1.0)
            sums_sb = const.tile([1, BT], f32, tag="sums_sb")

            def col(chunks, c, w):
                i, off = divmod(c, 1024)
                return chunks[i][:, off : off + w]

            # Q^T / K^T projections: W stationary, xt moving -> psum -> bf16
            # (cast on ACT: DVE is loaded with Schraudolph exp later).
            # Only the pieces needed by attention block 0's first pairs are
            # emitted up-front; the rest interleave into the attention loop.
            qt_c = [big.tile([128, 1024], bf16, tag=f"qt{c}", name=f"qt{c}")
                    for c in range(4)]
            kt_c = [big.tile([128, 1024], bf16, tag=f"kt{c}", name=f"kt{c}")
                    for c in range(4)]
            vt_c = [big.tile([128, 1024], bf16, tag=f"vt{c}", name=f"vt{c}")
                    for c in range(4)]
            # w8_c[i] = fp8(V_chunk @ Wu) for key chunks 4i..4i+3: folding Wu
            # into V lets the attention matmul produce out^T directly
            w8_c = [big.tile([128, 4, 128], fp8, tag=f"w8_{i}", name=f"w8_{i}")
                    for i in range(8)]

            def pcast(eng, dst, src):
                if eng == "act":
                    nc.scalar.activation(dst, src, Copy)
                else:
                    nc.vector.tensor_copy(dst, src)

            def emit_qk(w_sb, dst, c, eng, split=False):
                # each 512-half cast on a different engine so the psum slot
                # frees in ~half the single-engine cast time
                pp = ps.tile([128, 1024], f32, tag="s", name="pp")
                for g in range(2):
                    nc.tensor.matmul(
                        pp[:, 512 * g : 512 * (g + 1)],
                        w_sb[:],
                        xt_c[c][:, 512 * g : 512 * (g + 1)],
                        start=True,
                        stop=True,
                    )
                    pcast("act" if g == 0 else "vec",
                          dst[c][:, 512 * g : 512 * (g + 1)],
                          pp[:, 512 * g : 512 * (g + 1)])

            def emit_w8(i, eng):
                pw = ps.tile([128, 512], f32, tag="s", name="pw")
                for j in range(4):
                    ch = 4 * i + j
                    nc.tensor.matmul(
                        pw[:, 128 * j : 128 * (j + 1)],
                        col(vt_c, 128 * ch, 128),
                        wu_sb[:],
                        start=True,
                        stop=True,
                    )
                pcast("act", w8_c[i][:, 0:2, :],
                      pw[:, 0:256].rearrange("p (a b) -> p a b", a=2))
                pcast("vec", w8_c[i][:, 2:4, :],
                      pw[:, 256:512].rearrange("p (a b) -> p a b", a=2))

            emit_qk(wq_sb, qt_c, 0, "act", split=True)
            emit_qk(wk_sb, kt_c, 0, "act", split=True)
            emit_qk(wv_sb, vt_c, 0, "vec")
            emit_w8(0, "act")
            emit_w8(1, "vec")
            emit_qk(wq_sb, qt_c, 1, "vec")
            emit_qk(wk_sb, kt_c, 1, "act")
            emit_qk(wv_sb, vt_c, 1, "vec")
            emit_w8(2, "act")
            emit_w8(3, "vec")
            emit_qk(wq_sb, qt_c, 2, "act")
            emit_qk(wk_sb, kt_c, 2, "vec")
            emit_qk(wv_sb, vt_c, 2, "act")
            emit_w8(4, "vec")
            emit_w8(5, "act")
            emit_qk(wq_sb, qt_c, 3, "vec")
            emit_qk(wk_sb, kt_c, 3, "act")
            emit_qk(wv_sb, vt_c, 3, "vec")
            emit_w8(6, "act")
            emit_w8(7, "vec")
            proj_rest = []

            # attention: 4 blocks x 16 chunks, 3-deep software pipeline with
            # block tails deferred into the next block's first chunks
            chunks = [(blk, c) for blk in range(NBLK) for c in range(NCH)]
            DEPTH = 3

            def s_matmul(blk, c):
                b = blk // 2
                scol = b * T + c * 128
                tcol = blk * TB
                sp = ps.tile([128, TB], f32, tag="s", name=f"ps_{blk}_{c}")
                for g in range(2):
                    nc.tensor.matmul(
                        sp[:, 512 * g : 512 * (g + 1)],
                        col(kt_c, scol, 128),
                        col(qt_c, tcol + 512 * g, 512),
                        start=True,
                        stop=True,
                    )
                return sp

            def tail(blk, step, st):
                # deferred tail of block `blk`; step advances with the next
                # block's chunks so no engine queue blocks at the boundary
                if step == 0:
                    out_sb = work.tile([128, TB], f32, tag="outsb",
                                       name=f"osb{blk}")
                    nc.vector.tensor_copy(out_sb[:], st["py"][:])
                    nc.sync.dma_start(
                        out_d[:, blk * TB : (blk + 1) * TB], out_sb[:])
                elif step == 1:
                    psumt = ps.tile([128, TB], f32, tag="s", name=f"sum{blk}")
                    for pair in range(8):
                        for g in range(2):
                            sl = slice(512 * g, 512 * (g + 1))
                            nc.tensor.matmul(
                                psumt[:, sl],
                                ones8[:],
                                st["e8"][pair][:, :, sl],
                                start=(pair == 0),
                                stop=(pair == 7),
                                perf_mode=DR,
                            )
                    st["psumt"] = psumt
                elif step == 2:
                    nc.scalar.activation(
                        sums_sb[0:1, blk * TB : (blk + 1) * TB],
                        st["psumt"][0:1, :], Copy)

            pend = [s_matmul(*chunks[i]) for i in range(DEPTH)]
            prev_tail = None
            st = None
            pend_ydr = []

            def flush_one_ydr():
                yst, pair, vp, voff, e8t = pend_ydr.pop(0)
                for g in range(2):
                    sl = slice(512 * g, 512 * (g + 1))
                    nc.tensor.matmul(
                        yst["py"][:, sl],
                        vp[:, voff : voff + 2, :],
                        e8t[:, :, sl],
                        start=(pair == 0),
                        stop=(pair == 7),
                        perf_mode=DR,
                        skip_group_check=True,
                    )

            def flush_ydr():
                while pend_ydr:
                    flush_one_ydr()
            for ci, (blk, c) in enumerate(chunks):
                if c == 0:
                    st = {"py": ps_y.tile([128, TB], f32, tag="y",
                                          name=f"py{blk}"),
                          "e8": []}
                sp = pend.pop(0)
                if ci + DEPTH < len(chunks):
                    pend.append(s_matmul(*chunks[ci + DEPTH]))
                if c % 2 == 0:
                    st["e8"].append(e8p.tile([128, 2, TB], fp8, tag="e8",
                                             name=f"e8_{blk}_{c // 2}"))
                e8_pair = st["e8"][c // 2]
                if _is_dve_chunk(c):
                    nc.vector.tensor_scalar(
                        e8_pair[:, c % 2, :].bitcast(i8),
                        sp[:],
                        SCHR_A,
                        SCHR_B,
                        mybir.AluOpType.mult,
                        mybir.AluOpType.add,
                    )
                else:
                    nc.scalar.activation(
                        e8_pair[:, c % 2, :], sp[:], Exp, scale=float(SCALE)
                    )
                if c % 2 == 1:
                    pair = c // 2
                    b = blk // 2
                    vp = w8_c[(b * NCH + c - 1) // 4]
                    voff = ((c - 1) % 4) // 2 * 2
                    pend_ydr.append((st, pair, vp, voff, e8_pair))
                else:
                    while len(pend_ydr) > 1:
                        flush_one_ydr()
                if c % 2 == 1 and proj_rest:
                    proj_rest.pop(0)()
                if prev_tail is not None and c in (0, 1, 2):
                    if c == 0:
                        flush_ydr()
                    tail(prev_tail[0], c, prev_tail[1])
                    if c == 2:
                        prev_tail = None
                if c == NCH - 1:
                    prev_tail = (blk, st)
            # last block's tail: out copy + DMA first, sum burst last
            flush_ydr()
            lblk, lst = prev_tail
            out_sb = work.tile([128, TB], f32, tag="outsb", name="osbL")
            for g in range(2):
                sl = slice(512 * g, 512 * (g + 1))
                nc.vector.tensor_copy(out_sb[:, sl], lst["py"][:, sl])
                nc.sync.dma_start(out_d[:, lblk * TB + 512 * g :
                                        lblk * TB + 512 * (g + 1)],
                                  out_sb[:, sl])
            psumt = ps.tile([128, TB], f32, tag="s", name="sumL")
            for g in range(2):
                sl = slice(512 * g, 512 * (g + 1))
                for pair in range(8):
                    nc.tensor.matmul(
                        psumt[:, sl],
                        ones8[:],
                        lst["e8"][pair][:, :, sl],
                        start=(pair == 0),
                        stop=(pair == 7),
                        perf_mode=DR,
                    )
                nc.scalar.activation(
                    sums_sb[0:1, lblk * TB + 512 * g : lblk * TB + 512 * (g + 1)],
                    psumt[0:1, sl], Copy)
                nc.sync.dma_start(
                    sums_d[0:1, lblk * TB + 512 * g : lblk * TB + 512 * (g + 1)],
                    sums_sb[0:1, lblk * TB + 512 * g : lblk * TB + 512 * (g + 1)])
            nc.sync.dma_start(sums_d[0:1, 0 : lblk * TB],
                              sums_sb[0:1, 0 : lblk * TB])

    nc.compile()
    return nc


def _get_nc():
    global _compiled
    if _compiled is None:
        _compiled = _build()
    return _compiled


def kernel(x, Wq, Wk, Wv, Wu, bu, **_run_kwargs):
    from concourse.bass_utils import run_bass_kernel_spmd

    nc = _get_nc()

    bf = ml_dtypes.bfloat16
    x = np.asarray(x, dtype=np.float32).reshape(BT, K)
    xt = np.ascontiguousarray(x.T.astype(bf))
    Wq = np.asarray(Wq, dtype=np.float32)
    Wk = np.asarray(Wk, dtype=np.float32)
    Wv = np.asarray(Wv, dtype=np.float32)
    Wu = np.asarray(Wu, dtype=np.float32)
    bu = np.asarray(bu, dtype=np.float32)

    in_maps = []
    for c in range(NCORES):
        sl = slice(c * K, (c + 1) * K)
        in_maps.append(
            {
                "xt": xt,
                "wq": np.ascontiguousarray(Wq[:, sl].astype(bf)),
                "wk": np.ascontiguousarray(Wk[:, sl].astype(bf)),
                "wv": np.ascontiguousarray(Wv[:, sl].astype(bf)),
                "wu": np.ascontiguousarray(Wu[sl, :].astype(bf)),
            }
        )

    res = run_bass_kernel_spmd(nc, in_maps, list(range(NCORES)), **_run_kwargs)

    out = np.zeros((BT, K), dtype=np.float64)
    for c in range(NCORES):
        o = np.asarray(res.results[c]["out"], dtype=np.float64)   # [128, 4096]
        s = np.asarray(res.results[c]["sums"], dtype=np.float64).reshape(BT)
        out += (o / s[None, :]).T
    out += bu[None, :].astype(np.float64)
    result = out.astype(np.float32).reshape(B, T, K)
    if _run_kwargs:
        return result, res
    return result


# revision 21
# speedup vs baseline: 1.0120x; 1.0034x over previous
"""Multi-head attention (B=2, T=2048, H=8, K=128) on 8 TRN2 NeuronCores.

Sharding: tensor-parallel over heads - core c owns head c for both batches.
Each core computes its head's (unnormalized) attention output projected
through its slice of Wu; the host divides by the shipped per-token softmax
denominators, sums the 8 partials and adds the bias.

Per-core dataflow (features on partitions, tokens on the free axis):

  host:  xt = bf16(x^T) [128 i, 4096 t]; W slices pre-cast to bf16.
  qt/kt = Wq/Wk^T  xt     [128 d, 4096]  bf16  (W stationary, xt moving)
  V     = (xt_c)^T Wv     [128 keys, 128 d] per 128-token chunk - produced
          directly in [keys, dims] layout by making the xt chunk stationary;
          cast to fp8e4 (DVE) into DoubleRow slabs.
  per 1024-token block, per 128-key chunk c (3-deep software pipeline):
      S^T_c = kt_c qt       [128, 1024] PSUM fp32        (bf16 matmul)
      E_c   = exp(S^T_c/sqrt(128)) -> fp8e4 SBUF
              even chunks + every 8th odd: ACT Exp (direct fp8 output)
              other odd chunks: DVE Schraudolph - int8(S*a+b) IS the fp8
              bit pattern of exp (a=8 log2(e)/sqrt(128), b=56-0.3)
    per pair p (chunks 2p, 2p+1), fp8 DoubleRow matmul (0.5 cyc/row):
      Y^T += V_pair^T E_pair     [128, 1024] PSUM
    block tail (deferred into the next block's first chunks so no engine
    queue stalls at the boundary):
      sumexp = ones8^T E_pair, burst over all 8 pairs   (fp8 DR)
      y_bf = bf16(Y^T) (DVE); out^T_blk = Wu^T y_bf (bf16);
      out^T -> SBUF (DVE) -> DRAM; sumexp row 0 -> SBUF (ACT) -> DRAM.

Host: out = sum_c (out_c / sumexp_c)^T + bu, reshaped to (2, 2048, 128).

fp8 error budget (simulated offline vs fp64 truth): E+V fp8 with fp8-domain
Schraudolph on 7/16 of chunks -> rel err ~1.4e-2 (gate: 2e-2); softmax
normalization absorbs most of the correlated low-precision-E error.

PSUM (8 banks): ps pool 3x[128,1024]f32 (S tiles / sum burst / out proj,
rotating) = 6 banks, ps_y 1x[128,1024] (Y accumulator) = 2 banks.
"""

import sys

import numpy as np

if "/opt/trn_rl_repo" not in sys.path:
    sys.path.insert(0, "/opt/trn_rl_repo")

import ml_dtypes

B, T, K, H = 2, 2048, 128, 8
BT = B * T              # 4096 tokens over both batches
TB = 1024               # token block
NBLK = BT // TB         # 4
NCH = T // 128          # 16 key chunks per batch
NCORES = 8
SCALE = 1.0 / np.sqrt(np.float32(K))
SCHR_A = float(SCALE * np.log2(np.e) * 8.0)
SCHR_B = 56.0 - 0.3

_compiled = None


def _is_dve_chunk(c):
    # DVE (Schraudolph) takes odd chunks, ACT takes even -> no double-ACT
    # pairs to stall the pipeline
    return c % 2 == 1


def _build():
    import concourse.mybir as mybir
    import concourse.tile as tile
    from concourse import bacc

    f32 = mybir.dt.float32
    bf16 = mybir.dt.bfloat16
    fp8 = mybir.dt.float8e4
    i8 = mybir.dt.int8
    Exp = mybir.ActivationFunctionType.Exp
    Copy = mybir.ActivationFunctionType.Copy
    DR = mybir.MatmulPerfMode.DoubleRow

    nc = bacc.Bacc(
        "TRN2",
        target_bir_lowering=False,
        debug=False,
        enable_asserts=False,
        num_devices=NCORES,
    )

    xt_d = nc.dram_tensor("xt", [K, BT], bf16, kind="ExternalInput").ap()
    wq_d = nc.dram_tensor("wq", [K, K], bf16, kind="ExternalInput").ap()
    wk_d = nc.dram_tensor("wk", [K, K], bf16, kind="ExternalInput").ap()
    wv_d = nc.dram_tensor("wv", [K, K], bf16, kind="ExternalInput").ap()
    wu_d = nc.dram_tensor("wu", [K, K], bf16, kind="ExternalInput").ap()
    out_d = nc.dram_tensor("out", [K, BT], f32, kind="ExternalOutput").ap()
    sums_d = nc.dram_tensor("sums", [1, BT], f32, kind="ExternalOutput").ap()

    with tile.TileContext(nc) as tc:
        from contextlib import ExitStack

        with ExitStack() as ctx:
            const = ctx.enter_context(tc.tile_pool(name="const", bufs=1))
            big = ctx.enter_context(tc.tile_pool(name="big", bufs=1))
            e8p = ctx.enter_context(tc.tile_pool(name="e8p", bufs=11))
            work = ctx.enter_context(tc.tile_pool(name="work", bufs=2))
            ps = ctx.enter_context(tc.tile_pool(name="ps", bufs=3, space="PSUM"))
            ps_y = ctx.enter_context(tc.tile_pool(name="ps_y", bufs=1, space="PSUM"))

            # inputs: xt chunk 0 + weights first (gate the projections), xt
            # rest on the idle gpsimd queue so the DMAs issue in parallel
            xt_c = [big.tile([128, 1024], bf16, tag=f"xt{c}", name=f"xt{c}")
                    for c in range(4)]
            wq_sb = const.tile([128, 128], bf16, tag="wq")
            wk_sb = const.tile([128, 128], bf16, tag="wk")
            wv_sb = const.tile([128, 128], bf16, tag="wv")
            wu_sb = const.tile([128, 128], bf16, tag="wu")
            nc.sync.dma_start(xt_c[0][:], xt_d[:, 0:1024])
            nc.sync.dma_start(wq_sb[:], wq_d[:])
            nc.scalar.dma_start(wk_sb[:], wk_d[:])
            nc.scalar.dma_start(wv_sb[:], wv_d[:])
            nc.scalar.dma_start(wu_sb[:], wu_d[:])
            for c in range(1, 4):
                nc.gpsimd.dma_start(xt_c[c][:], xt_d[:, 1024 * c : 1024 * (c + 1)])

            ones8 = const.tile([128, 2, 128], fp8, tag="ones8")
            nc.gpsimd.memset(ones8[:], 1.0)
            sums_sb = const.tile([1, BT], f32, tag="sums_sb")

            def col(chunks, c, w):
                i, off = divmod(c, 1024)
                return chunks[i][:, off : off + w]

            # Q^T / K^T projections: W stationary, xt moving -> psum -> bf16
            # (cast on ACT: DVE is loaded with Schraudolph exp later).
            # Only the pieces needed by attention block 0's first pairs are
            # emitted up-front; the rest interleave into the attention loop.
            qt_c = [big.tile([128, 1024], bf16, tag=f"qt{c}", name=f"qt{c}")
                    for c in range(4)]
            kt_c = [big.tile([128, 1024], bf16, tag=f"kt{c}", name=f"kt{c}")
                    for c in range(4)]
            vt_c = [big.tile([128, 1024], bf16, tag=f"vt{c}", name=f"vt{c}")
                    for c in range(4)]
            # w8_c[i] = fp8(V_chunk @ Wu) for key chunks 4i..4i+3: folding Wu
            # into V lets the attention matmul produce out^T directly
            w8_c = [big.tile([128, 4, 128], fp8, tag=f"w8_{i}", name=f"w8_{i}")
                    for i in range(8)]

            def pcast(eng, dst, src):
                if eng == "act":
                    nc.scalar.activation(dst, src, Copy)
                else:
                    nc.vector.tensor_copy(dst, src)

            def emit_qk(w_sb, dst, c, eng, split=False):
                pp = ps.tile([128, 1024], f32, tag="s", name="pp")
                for g in range(2):
                    nc.tensor.matmul(
                        pp[:, 512 * g : 512 * (g + 1)],
                        w_sb[:],
                        xt_c[c][:, 512 * g : 512 * (g + 1)],
                        start=True,
                        stop=True,
                    )
                    if split:
                        pcast(eng, dst[c][:, 512 * g : 512 * (g + 1)],
                              pp[:, 512 * g : 512 * (g + 1)])
                if not split:
                    pcast(eng, dst[c][:], pp[:])

            def emit_w8(i, eng):
                pw = ps.tile([128, 512], f32, tag="s", name="pw")
                for j in range(4):
                    ch = 4 * i + j
                    nc.tensor.matmul(
                        pw[:, 128 * j : 128 * (j + 1)],
                        col(vt_c, 128 * ch, 128),
                        wu_sb[:],
                        start=True,
                        stop=True,
                    )
                pcast(eng, w8_c[i][:],
                      pw[:].rearrange("p (a b) -> p a b", a=4))

            emit_qk(wq_sb, qt_c, 0, "act", split=True)
            emit_qk(wk_sb, kt_c, 0, "act", split=True)
            emit_qk(wv_sb, vt_c, 0, "vec")
            emit_w8(0, "act")
            emit_w8(1, "vec")
            emit_qk(wq_sb, qt_c, 1, "vec")
            emit_qk(wk_sb, kt_c, 1, "act")
            emit_qk(wv_sb, vt_c, 1, "vec")
            emit_w8(2, "act")
            emit_w8(3, "vec")
            emit_qk(wq_sb, qt_c, 2, "act")
            emit_qk(wk_sb, kt_c, 2, "vec")
            emit_qk(wv_sb, vt_c, 2, "act")
            emit_w8(4, "vec")
            emit_w8(5, "act")
            emit_qk(wq_sb, qt_c, 3, "vec")
            emit_qk(wk_sb, kt_c, 3, "act")
            emit_qk(wv_sb, vt_c, 3, "vec")
            emit_w8(6, "act")
            emit_w8(7, "vec")
            proj_rest = []

            # attention: 4 blocks x 16 chunks, 3-deep software pipeline with
            # block tails deferred into the next block's first chunks
            chunks = [(blk, c) for blk in range(NBLK) for c in range(NCH)]
            DEPTH = 3

            def s_matmul(blk, c):
                b = blk // 2
                scol = b * T + c * 128
                tcol = blk * TB
                sp = ps.tile([128, TB], f32, tag="s", name=f"ps_{blk}_{c}")
                for g in range(2):
                    nc.tensor.matmul(
                        sp[:, 512 * g : 512 * (g + 1)],
                        col(kt_c, scol, 128),
                        col(qt_c, tcol + 512 * g, 512),
                        start=True,
                        stop=True,
                    )
                return sp

            def tail(blk, step, st):
                # deferred tail of block `blk`; step advances with the next
                # block's chunks so no engine queue blocks at the boundary
                if step == 0:
                    out_sb = work.tile([128, TB], f32, tag="outsb",
                                       name=f"osb{blk}")
                    nc.vector.tensor_copy(out_sb[:], st["py"][:])
                    nc.sync.dma_start(
                        out_d[:, blk * TB : (blk + 1) * TB], out_sb[:])
                elif step == 1:
                    psumt = ps.tile([128, TB], f32, tag="s", name=f"sum{blk}")
                    for pair in range(8):
                        for g in range(2):
                            sl = slice(512 * g, 512 * (g + 1))
                            nc.tensor.matmul(
                                psumt[:, sl],
                                ones8[:],
                                st["e8"][pair][:, :, sl],
                                start=(pair == 0),
                                stop=(pair == 7),
                                perf_mode=DR,
                            )
                    st["psumt"] = psumt
                elif step == 2:
                    nc.scalar.activation(
                        sums_sb[0:1, blk * TB : (blk + 1) * TB],
                        st["psumt"][0:1, :], Copy)

            pend = [s_matmul(*chunks[i]) for i in range(DEPTH)]
            prev_tail = None
            st = None
            pend_ydr = []

            def flush_one_ydr():
                yst, pair, vp, voff, e8t = pend_ydr.pop(0)
                for g in range(2):
                    sl = slice(512 * g, 512 * (g + 1))
                    nc.tensor.matmul(
                        yst["py"][:, sl],
                        vp[:, voff : voff + 2, :],
                        e8t[:, :, sl],
                        start=(pair == 0),
                        stop=(pair == 7),
                        perf_mode=DR,
                        skip_group_check=True,
                    )

            def flush_ydr():
                while pend_ydr:
                    flush_one_ydr()
            for ci, (blk, c) in enumerate(chunks):
                if c == 0:
                    st = {"py": ps_y.tile([128, TB], f32, tag="y",
                                          name=f"py{blk}"),
                          "e8": []}
                sp = pend.pop(0)
                if ci + DEPTH < len(chunks):
                    pend.append(s_matmul(*chunks[ci + DEPTH]))
                if c % 2 == 0:
                    st["e8"].append(e8p.tile([128, 2, TB], fp8, tag="e8",
                                             name=f"e8_{blk}_{c // 2}"))
                e8_pair = st["e8"][c // 2]
                if _is_dve_chunk(c):
                    nc.vector.tensor_scalar(
                        e8_pair[:, c % 2, :].bitcast(i8),
                        sp[:],
                        SCHR_A,
                        SCHR_B,
                        mybir.AluOpType.mult,
                        mybir.AluOpType.add,
                    )
                else:
                    nc.scalar.activation(
                        e8_pair[:, c % 2, :], sp[:], Exp, scale=float(SCALE)
                    )
                if c % 2 == 1:
                    pair = c // 2
                    b = blk // 2
                    vp = w8_c[(b * NCH + c - 1) // 4]
                    voff = ((c - 1) % 4) // 2 * 2
                    pend_ydr.append((st, pair, vp, voff, e8_pair))
                else:
                    while len(pend_ydr) > 1:
                        flush_one_ydr()
                if c % 2 == 1 and proj_rest:
                    proj_rest.pop(0)()
                if prev_tail is not None and c in (0, 1, 2):
                    if c == 0:
                        flush_ydr()
                    tail(prev_tail[0], c, prev_tail[1])
                    if c == 2:
                        prev_tail = None
                if c == NCH - 1:
                    prev_tail = (blk, st)
            # last block's tail: out copy + DMA first, sum burst last
            flush_ydr()
            lblk, lst = prev_tail
            out_sb = work.tile([128, TB], f32, tag="outsb", name="osbL")
            for g in range(2):
                sl = slice(512 * g, 512 * (g + 1))
                nc.vector.tensor_copy(out_sb[:, sl], lst["py"][:, sl])
                nc.sync.dma_start(out_d[:, lblk * TB + 512 * g :
                                        lblk * TB + 512 * (g + 1)],
                                  out_sb[:, sl])
            psumt = ps.tile([128, TB], f32, tag="s", name="sumL")
            for g in range(2):
                sl = slice(512 * g, 512 * (g + 1))
                for pair in range(8):
                    nc.tensor.matmul(
                        psumt[:, sl],
                        ones8[:],
                        lst["e8"][pair][:, :, sl],
                        start=(pair == 0),
                        stop=(pair == 7),
                        perf_mode=DR,
                    )
                nc.scalar.activation(
                    sums_sb[0:1, lblk * TB + 512 * g : lblk * TB + 512 * (g + 1)],
                    psumt[0:1, sl], Copy)
                nc.sync.dma_start(
                    sums_d[0:1, lblk * TB + 512 * g : lblk * TB + 512 * (g + 1)],
                    sums_sb[0:1, lblk * TB + 512 * g : lblk * TB + 512 * (g + 1)])
            nc.sync.dma_start(sums_d[0:1, 0 : lblk * TB],
                              sums_sb[0:1, 0 : lblk * TB])

    nc.compile()
    return nc


def _get_nc():
    global _compiled
    if _compiled is None:
        _compiled = _build()
    return _compiled


def kernel(x, Wq, Wk, Wv, Wu, bu, **_run_kwargs):
    from concourse.bass_utils import run_bass_kernel_spmd

    nc = _get_nc()

    bf = ml_dtypes.bfloat16
    x = np.asarray(x, dtype=np.float32).reshape(BT, K)
    xt = np.ascontiguousarray(x.T.astype(bf))
    Wq = np.asarray(Wq, dtype=np.float32)
    Wk = np.asarray(Wk, dtype=np.float32)
    Wv = np.asarray(Wv, dtype=np.float32)
    Wu = np.asarray(Wu, dtype=np.float32)
    bu = np.asarray(bu, dtype=np.float32)

    in_maps = []
    for c in range(NCORES):
        sl = slice(c * K, (c + 1) * K)
        in_maps.append(
            {
                "xt": xt,
                "wq": np.ascontiguousarray(Wq[:, sl].astype(bf)),
                "wk": np.ascontiguousarray(Wk[:, sl].astype(bf)),
                "wv": np.ascontiguousarray(Wv[:, sl].astype(bf)),
                "wu": np.ascontiguousarray(Wu[sl, :].astype(bf)),
            }
        )

    res = run_bass_kernel_spmd(nc, in_maps, list(range(NCORES)), **_run_kwargs)

    out = np.zeros((BT, K), dtype=np.float64)
    for c in range(NCORES):
        o = np.asarray(res.results[c]["out"], dtype=np.float64)   # [128, 4096]
        s = np.asarray(res.results[c]["sums"], dtype=np.float64).reshape(BT)
        out += (o / s[None, :]).T
    out += bu[None, :].astype(np.float64)
    result = out.astype(np.float32).reshape(B, T, K)
    if _run_kwargs:
        return result, res
    return result
